# revision 2
# baseline (speedup 1.0000x reference)
"""Alpha-filter (keras_spiking AlphaCell) Trainium2 Bass kernel.

Math: per (batch b, feature k) the reference runs the 2-state recurrence
    x_t = A_k x_{t-1} + B_k u_t,   y_t = x_t[1]
with A_k = e*[[1-a, -a/tau],[dt, 1+a]], a = dt/tau, e = exp(-a), which
reduces to two chained first-order scans (defective double pole at e):
    s_t = e s_{t-1} + u_t;  eta_t = e eta_{t-1} + s_{t-1};
    y_t = c_e eta_t + c_s s_t,   c_e = e*a*(1-e), c_s = (1-e)-e*a.

Instead of running the scans on the (slow, ~2.2 cycles/element) DVE scan
unit, time is split into chunks of C=128 and the chunk-local response is
computed on the PE as two shared-weight triangular matmuls via an
exponential rescale: with uhat_j = u_j * e^{-j} (per-feature prescale),

    P_t = sum_{j<=t} uhat_j + BA * sum_{j<=t} (t-j) uhat_j,
    y_local_t = c_s e^t P_t,           BA = c_e/(e c_s)

where the two lower-triangular weight matrices (ones / ramp) are shared
by every feature.  The device computes P for all chunks:
  DMA-in (bf16) -> DVE: U2 = U*BA (one 2-byte 2x op) ->
  PE: P = W1^T@U + W2^T@U2 (PSUM fp32, 512-col banks) ->
  Scalar: PSUM->SBUF bf16 -> DMA-out.
The host (numpy) performs the layout/prescale on the way in, and on the
way out applies the closed-form e^t/c_s post-scale plus the exact
cross-chunk boundary corrections (boundary states computed host-side in
fp64 from per-chunk weighted sums; all O(B*T*K) elementwise work, the
O(B*T*K*C) convolution itself runs on device).  bf16 I/O halves HBM
traffic; validated end-to-end rel err ~5e-3 (gate 2e-2).

Sharding: data-parallel over batch, 8 batches per core x 8 cores.
"""

import sys

for _p in ("/opt/trn_rl_repo",):
    if _p not in sys.path:
        sys.path.insert(0, _p)

from contextlib import ExitStack

import ml_dtypes
import numpy as np

import concourse.bacc as bacc
import concourse.bass as bass
import concourse.tile as tile
from concourse import mybir
from concourse.bass_utils import run_bass_kernel_spmd

DT = 0.001
B, T, K = 64, 1024, 512
N_CORES = 8
B_LOC = B // N_CORES  # 8 batches per core
P = 128               # chunk length == partition count
NC = T // P           # 8 time chunks
COLS = NC * K         # 4096 free columns per batch
HALF = COLS // 2      # 2048-col PSUM tiles (4 banks)
QUAD = 512            # one PSUM bank / max moving free dim

F32 = mybir.dt.float32
BF16 = mybir.dt.bfloat16
MULT = mybir.AluOpType.mult

BF = ml_dtypes.bfloat16


def build_nc():
    nc = bacc.Bacc(None, target_bir_lowering=False)

    xs = nc.dram_tensor("xs", [B_LOC, P, COLS], BF16, kind="ExternalInput")
    ba = nc.dram_tensor("ba", [P, COLS], BF16, kind="ExternalInput")
    w1 = nc.dram_tensor("w1", [P, P], BF16, kind="ExternalInput")
    w2 = nc.dram_tensor("w2", [P, P], BF16, kind="ExternalInput")
    ys = nc.dram_tensor("ys", [B_LOC, P, COLS], BF16, kind="ExternalOutput")

    with tile.TileContext(nc) as tc, ExitStack() as ctx:
        singles = ctx.enter_context(tc.tile_pool(name="singles", bufs=1))
        inpool = ctx.enter_context(tc.tile_pool(name="inpool", bufs=3))
        u2pool = ctx.enter_context(tc.tile_pool(name="u2pool", bufs=2))
        outpool = ctx.enter_context(tc.tile_pool(name="outpool", bufs=2))
        psum = ctx.enter_context(tc.tile_pool(name="psum", bufs=2, space="PSUM"))

        # one-time constants
        w1_t = singles.tile([P, P], BF16)
        w2_t = singles.tile([P, P], BF16)
        ba_t = singles.tile([P, COLS], BF16)
        nc.sync.dma_start(out=w1_t[:], in_=w1[:])
        nc.sync.dma_start(out=w2_t[:], in_=w2[:])
        nc.scalar.dma_start(out=ba_t[:], in_=ba[:])

        # PE warm-up during the initial DMA window: HAM needs ~3.4us of
        # activity before the PE clock reaches full rate.
        scratch = singles.tile([P, P], BF16)
        nc.gpsimd.memset(scratch[:], 0.0)
        warm = psum.tile([P, HALF], F32, name="warm", tag="ph")
        for _ in range(6):
            nc.tensor.matmul(warm[:, 0:P], scratch[:], scratch[:], start=True, stop=True)

        for b in range(B_LOC):
            U = inpool.tile([P, COLS], BF16)
            for h in range(2):
                eng = (nc.sync, nc.scalar)[h]
                eng.dma_start(
                    out=U[:, h * HALF : (h + 1) * HALF],
                    in_=xs[b][:, h * HALF : (h + 1) * HALF],
                )
            U2 = u2pool.tile([P, COLS], BF16)
            nc.vector.tensor_tensor(out=U2[:], in0=U[:], in1=ba_t[:], op=MULT)

            Y = outpool.tile([P, COLS], BF16)
            for h in range(2):
                Ph = psum.tile([P, HALF], F32, tag="ph")
                base = h * HALF
                for q in range(4):
                    nc.tensor.matmul(
                        Ph[:, q * QUAD : (q + 1) * QUAD],
                        w1_t[:],
                        U[:, base + q * QUAD : base + (q + 1) * QUAD],
                        start=True,
                        stop=False,
                    )
                for q in range(4):
                    nc.tensor.matmul(
                        Ph[:, q * QUAD : (q + 1) * QUAD],
                        w2_t[:],
                        U2[:, base + q * QUAD : base + (q + 1) * QUAD],
                        start=False,
                        stop=True,
                    )
                nc.scalar.copy(Y[:, base : base + HALF], Ph[:])
                eng = (nc.sync, nc.scalar)[h]
                eng.dma_start(
                    out=ys[b][:, base : base + HALF],
                    in_=Y[:, base : base + HALF],
                )

    nc.compile()
    return nc


_CACHE = {}
PROFILE = False
LAST_RESULT = None


def _constants(initial_level, tau):
    tau_c = np.maximum(tau.astype(np.float64), 1e-8)
    a = DT / tau_c
    e = np.exp(-a)
    c_e = e * a * (1.0 - e)
    c_s = (1.0 - e) - e * a
    return e, c_e, c_s


def kernel(inputs, initial_level, tau):
    global LAST_RESULT
    inputs = np.asarray(inputs, dtype=np.float32)
    initial_level = np.asarray(initial_level, dtype=np.float32)
    tau = np.asarray(tau, dtype=np.float32)
    assert inputs.shape == (B, T, K), inputs.shape

    e, c_e, c_s = _constants(initial_level, tau)
    BA = c_e / (e * c_s)
    j = np.arange(P)
    e_mj = (e[None, :] ** (-j[:, None])).astype(np.float32)   # [P, K]
    e_t = e[None, :] ** (j[:, None])                          # [P, K] fp64

    # ---- prescale + chunk layout: xs[b, j, c*K+k] = u[b, c*P+j, k]*e^-j
    u4 = inputs.reshape(B, NC, P, K)
    xs = np.ascontiguousarray(
        (u4 * e_mj[None, None]).transpose(0, 2, 1, 3)
    ).reshape(B, P, COLS).astype(BF)

    # ---- boundary states (host, fp64; exact cross-chunk stitching) ----
    ws = e[None, :] ** (P - 1 - j[:, None])                   # [P, K]
    weta = (P - 1 - j)[:, None] * e[None, :] ** np.minimum(
        P - 2 - j[:, None], P
    )  # (P-1-j)*e^{P-2-j}; j=P-1 term is zero anyway
    weta[P - 1, :] = 0.0
    inj_s = np.einsum("jk,bcjk->bck", ws, u4, optimize=True)
    inj_eta = np.einsum("jk,bcjk->bck", weta, u4, optimize=True)
    eC = e**P
    s_b = np.broadcast_to(initial_level.astype(np.float64) / (1 - e), (B, K)).copy()
    eta_b = np.broadcast_to(
        initial_level.astype(np.float64) / (1 - e) ** 2, (B, K)
    ).copy()
    s_all = np.empty((B, NC, K))
    eta_all = np.empty((B, NC, K))
    for c in range(NC):
        s_all[:, c] = s_b
        eta_all[:, c] = eta_b
        s_new = eC[None, :] * s_b + inj_s[:, c]
        eta_b = eC[None, :] * eta_b + P * (e ** (P - 1))[None, :] * s_b + inj_eta[:, c]
        s_b = s_new

    # ---- device constants ----
    ba_mat = np.tile(BA.astype(BF)[None, :], (P, NC))         # [P, COLS]
    tri1 = (j[:, None] <= j[None, :]).astype(BF)              # W1[j, t]
    tri2 = (np.maximum(j[None, :] - j[:, None], 0)).astype(BF)  # W2[j, t]

    if "nc" not in _CACHE:
        _CACHE["nc"] = build_nc()
    nc = _CACHE["nc"]

    in_maps = [
        {
            "xs": xs[i * B_LOC : (i + 1) * B_LOC],
            "ba": ba_mat,
            "w1": tri1,
            "w2": tri2,
        }
        for i in range(N_CORES)
    ]
    res = run_bass_kernel_spmd(nc, in_maps, list(range(N_CORES)), trace=PROFILE)
    LAST_RESULT = res
    P_dev = np.concatenate([np.asarray(r["ys"]) for r in res.results], axis=0)

    # ---- host post: y = e^t*(c_s*P + c_e*e*eta_b) + t1*s_b ----
    Pf = (
        P_dev.astype(np.float32)
        .reshape(B, P, NC, K)
        .transpose(0, 2, 1, 3)
    )  # [B, NC, P(t), K]
    e_t32 = e_t.astype(np.float32)
    t1 = ((c_s * e)[None, :] * e_t + c_e[None, :] * (j + 1)[:, None] * e_t).astype(
        np.float32
    )  # [P, K]
    cee = (c_e * e)[None, :] * e_t                             # [P, K] fp64
    y = (
        e_t32[None, None] * (c_s.astype(np.float32)[None, None, None] * Pf)
        + cee.astype(np.float32)[None, None] * eta_all.astype(np.float32)[:, :, None, :]
        + t1[None, None] * s_all.astype(np.float32)[:, :, None, :]
    )
    return np.ascontiguousarray(y.reshape(B, T, K), dtype=np.float32)


# revision 4
# speedup vs baseline: 1.1136x; 1.1136x over previous
"""Alpha-filter (keras_spiking AlphaCell) Trainium2 Bass kernel.

Math: per (batch b, feature k) the reference runs the 2-state recurrence
    x_t = A_k x_{t-1} + B_k u_t,   y_t = x_t[1]
with A_k = e*[[1-a, -a/tau],[dt, 1+a]], a = dt/tau, e = exp(-a), which
reduces to two chained first-order scans (defective double pole at e):
    s_t = e s_{t-1} + u_t;  eta_t = e eta_{t-1} + s_{t-1};
    y_t = c_e eta_t + c_s s_t,   c_e = e*a*(1-e), c_s = (1-e)-e*a.

Instead of running the scans on the (slow, ~2.2 cycles/element) DVE scan
unit, time is split into chunks of C=128 and the chunk-local response is
computed on the PE as two shared-weight triangular matmuls via an
exponential rescale: with uhat_j = u_j * e^{-j} (per-feature prescale),

    P_t = sum_{j<=t} uhat_j + BA * sum_{j<=t} (t-j) uhat_j,
    y_local_t = c_s e^t P_t,           BA = c_e/(e c_s)

where the two lower-triangular weight matrices (ones / ramp) are shared
by every feature.  The device computes P for all chunks:
  DMA-in (bf16) -> DVE: U2 = U*BA (one 2-byte 2x op) ->
  PE: P = W1^T@U + W2^T@U2 (PSUM fp32, 512-col banks) ->
  Scalar: PSUM->SBUF bf16 -> DMA-out.
The host (numpy) performs the layout/prescale on the way in, and on the
way out applies the closed-form e^t/c_s post-scale plus the exact
cross-chunk boundary corrections (boundary states computed host-side in
fp64 from per-chunk weighted sums; all O(B*T*K) elementwise work, the
O(B*T*K*C) convolution itself runs on device).  bf16 I/O halves HBM
traffic; validated end-to-end rel err ~5e-3 (gate 2e-2).

Sharding: data-parallel over batch, 8 batches per core x 8 cores.
"""

import sys

for _p in ("/opt/trn_rl_repo",):
    if _p not in sys.path:
        sys.path.insert(0, _p)

from contextlib import ExitStack

import ml_dtypes
import numpy as np

import concourse.bacc as bacc
import concourse.bass as bass
import concourse.tile as tile
from concourse import mybir
from concourse.bass_utils import run_bass_kernel_spmd

DT = 0.001
B, T, K = 64, 1024, 512
N_CORES = 8
B_LOC = B // N_CORES  # 8 batches per core
P = 128               # chunk length == partition count
NC = T // P           # 8 time chunks
COLS = NC * K         # 4096 free columns per batch
HALF = COLS // 2      # 2048-col PSUM tiles (4 banks)
QUAD = 512            # one PSUM bank / max moving free dim

F32 = mybir.dt.float32
BF16 = mybir.dt.bfloat16
MULT = mybir.AluOpType.mult

BF = ml_dtypes.bfloat16


def build_nc():
    nc = bacc.Bacc(None, target_bir_lowering=False)

    xs = nc.dram_tensor("xs", [B_LOC, P, COLS], BF16, kind="ExternalInput")
    ba = nc.dram_tensor("ba", [P, COLS], BF16, kind="ExternalInput")
    w1 = nc.dram_tensor("w1", [P, P], BF16, kind="ExternalInput")
    w2 = nc.dram_tensor("w2", [P, P], BF16, kind="ExternalInput")
    ys = nc.dram_tensor("ys", [B_LOC, P, COLS], BF16, kind="ExternalOutput")

    with tile.TileContext(nc) as tc, ExitStack() as ctx:
        singles = ctx.enter_context(tc.tile_pool(name="singles", bufs=1))
        inpool = ctx.enter_context(tc.tile_pool(name="inpool", bufs=4))
        u2pool = ctx.enter_context(tc.tile_pool(name="u2pool", bufs=2))
        outpool = ctx.enter_context(tc.tile_pool(name="outpool", bufs=2))
        psum = ctx.enter_context(tc.tile_pool(name="psum", bufs=2, space="PSUM"))

        # one-time constants
        w1_t = singles.tile([P, P], BF16)
        w2_t = singles.tile([P, P], BF16)
        ba_t = singles.tile([P, COLS], BF16)
        nc.sync.dma_start(out=w1_t[:], in_=w1[:])
        nc.sync.dma_start(out=w2_t[:], in_=w2[:])
        nc.scalar.dma_start(out=ba_t[:], in_=ba[:])

        # PE warm-up during the initial DMA window: HAM needs ~3.4us of
        # activity before the PE clock reaches full rate.
        scratch = singles.tile([P, P], BF16)
        nc.gpsimd.memset(scratch[:], 0.0)
        warm = psum.tile([P, HALF], F32, name="warm", tag="ph")
        for _ in range(6):
            nc.tensor.matmul(warm[:, 0:P], scratch[:], scratch[:], start=True, stop=True)

        for b in range(B_LOC):
            U = inpool.tile([P, COLS], BF16)
            for h in range(2):
                eng = (nc.sync, nc.scalar)[h]
                eng.dma_start(
                    out=U[:, h * HALF : (h + 1) * HALF],
                    in_=xs[b][:, h * HALF : (h + 1) * HALF],
                )
            U2 = u2pool.tile([P, COLS], BF16)

            Y = outpool.tile([P, COLS], BF16)
            for h in range(2):
                base = h * HALF
                nc.vector.tensor_tensor(
                    out=U2[:, base : base + HALF],
                    in0=U[:, base : base + HALF],
                    in1=ba_t[:, base : base + HALF],
                    op=MULT,
                )
                Ph = psum.tile([P, HALF], F32, tag="ph")
                for q in range(4):
                    nc.tensor.matmul(
                        Ph[:, q * QUAD : (q + 1) * QUAD],
                        w1_t[:],
                        U[:, base + q * QUAD : base + (q + 1) * QUAD],
                        start=True,
                        stop=False,
                    )
                for q in range(4):
                    nc.tensor.matmul(
                        Ph[:, q * QUAD : (q + 1) * QUAD],
                        w2_t[:],
                        U2[:, base + q * QUAD : base + (q + 1) * QUAD],
                        start=False,
                        stop=True,
                    )
                if h == 0:
                    nc.scalar.copy(Y[:, base : base + HALF], Ph[:])
                else:
                    nc.vector.tensor_copy(Y[:, base : base + HALF], Ph[:])
                eng = (nc.sync, nc.scalar)[h]
                eng.dma_start(
                    out=ys[b][:, base : base + HALF],
                    in_=Y[:, base : base + HALF],
                )

    nc.compile()
    return nc


_CACHE = {}
PROFILE = False
LAST_RESULT = None


def _constants(initial_level, tau):
    tau_c = np.maximum(tau.astype(np.float64), 1e-8)
    a = DT / tau_c
    e = np.exp(-a)
    c_e = e * a * (1.0 - e)
    c_s = (1.0 - e) - e * a
    return e, c_e, c_s


def kernel(inputs, initial_level, tau):
    global LAST_RESULT
    inputs = np.asarray(inputs, dtype=np.float32)
    initial_level = np.asarray(initial_level, dtype=np.float32)
    tau = np.asarray(tau, dtype=np.float32)
    assert inputs.shape == (B, T, K), inputs.shape

    e, c_e, c_s = _constants(initial_level, tau)
    BA = c_e / (e * c_s)
    j = np.arange(P)
    e_mj = (e[None, :] ** (-j[:, None])).astype(np.float32)   # [P, K]
    e_t = e[None, :] ** (j[:, None])                          # [P, K] fp64

    # ---- prescale + chunk layout: xs[b, j, c*K+k] = u[b, c*P+j, k]*e^-j
    u4 = inputs.reshape(B, NC, P, K)
    xs = np.ascontiguousarray(
        (u4 * e_mj[None, None]).transpose(0, 2, 1, 3)
    ).reshape(B, P, COLS).astype(BF)

    # ---- boundary states (host, fp64; exact cross-chunk stitching) ----
    ws = e[None, :] ** (P - 1 - j[:, None])                   # [P, K]
    weta = (P - 1 - j)[:, None] * e[None, :] ** np.minimum(
        P - 2 - j[:, None], P
    )  # (P-1-j)*e^{P-2-j}; j=P-1 term is zero anyway
    weta[P - 1, :] = 0.0
    inj_s = np.einsum("jk,bcjk->bck", ws, u4, optimize=True)
    inj_eta = np.einsum("jk,bcjk->bck", weta, u4, optimize=True)
    eC = e**P
    s_b = np.broadcast_to(initial_level.astype(np.float64) / (1 - e), (B, K)).copy()
    eta_b = np.broadcast_to(
        initial_level.astype(np.float64) / (1 - e) ** 2, (B, K)
    ).copy()
    s_all = np.empty((B, NC, K))
    eta_all = np.empty((B, NC, K))
    for c in range(NC):
        s_all[:, c] = s_b
        eta_all[:, c] = eta_b
        s_new = eC[None, :] * s_b + inj_s[:, c]
        eta_b = eC[None, :] * eta_b + P * (e ** (P - 1))[None, :] * s_b + inj_eta[:, c]
        s_b = s_new

    # ---- device constants ----
    ba_mat = np.tile(BA.astype(BF)[None, :], (P, NC))         # [P, COLS]
    tri1 = (j[:, None] <= j[None, :]).astype(BF)              # W1[j, t]
    tri2 = (np.maximum(j[None, :] - j[:, None], 0)).astype(BF)  # W2[j, t]

    if "nc" not in _CACHE:
        _CACHE["nc"] = build_nc()
    nc = _CACHE["nc"]

    in_maps = [
        {
            "xs": xs[i * B_LOC : (i + 1) * B_LOC],
            "ba": ba_mat,
            "w1": tri1,
            "w2": tri2,
        }
        for i in range(N_CORES)
    ]
    res = run_bass_kernel_spmd(nc, in_maps, list(range(N_CORES)), trace=PROFILE)
    LAST_RESULT = res
    P_dev = np.concatenate([np.asarray(r["ys"]) for r in res.results], axis=0)

    # ---- host post: y = e^t*(c_s*P + c_e*e*eta_b) + t1*s_b ----
    Pf = (
        P_dev.astype(np.float32)
        .reshape(B, P, NC, K)
        .transpose(0, 2, 1, 3)
    )  # [B, NC, P(t), K]
    e_t32 = e_t.astype(np.float32)
    t1 = ((c_s * e)[None, :] * e_t + c_e[None, :] * (j + 1)[:, None] * e_t).astype(
        np.float32
    )  # [P, K]
    cee = (c_e * e)[None, :] * e_t                             # [P, K] fp64
    y = (
        e_t32[None, None] * (c_s.astype(np.float32)[None, None, None] * Pf)
        + cee.astype(np.float32)[None, None] * eta_all.astype(np.float32)[:, :, None, :]
        + t1[None, None] * s_all.astype(np.float32)[:, :, None, :]
    )
    return np.ascontiguousarray(y.reshape(B, T, K), dtype=np.float32)


# revision 8
# speedup vs baseline: 1.1290x; 1.0139x over previous
"""Alpha-filter (keras_spiking AlphaCell) Trainium2 Bass kernel.

Math: per (batch b, feature k) the reference runs the 2-state recurrence
    x_t = A_k x_{t-1} + B_k u_t,   y_t = x_t[1]
with A_k = e*[[1-a, -a/tau],[dt, 1+a]], a = dt/tau, e = exp(-a), which
reduces to two chained first-order scans (defective double pole at e):
    s_t = e s_{t-1} + u_t;  eta_t = e eta_{t-1} + s_{t-1};
    y_t = c_e eta_t + c_s s_t,   c_e = e*a*(1-e), c_s = (1-e)-e*a.

Instead of running the scans on the (slow, ~2.2 cycles/element) DVE scan
unit, time is split into chunks of C=128 and the chunk-local response is
computed on the PE as two shared-weight triangular matmuls via an
exponential rescale: with uhat_j = u_j * e^{-j} (per-feature prescale),

    P_t = sum_{j<=t} uhat_j + BA * sum_{j<=t} (t-j) uhat_j,
    y_local_t = c_s e^t P_t,           BA = c_e/(e c_s)

where the two lower-triangular weight matrices (ones / ramp) are shared
by every feature.  The device computes P for all chunks:
  DMA-in (bf16) -> DVE: U2 = U*BA (one 2-byte 2x op) ->
  PE: P = W1^T@U + W2^T@U2 (PSUM fp32, 512-col banks) ->
  Scalar: PSUM->SBUF bf16 -> DMA-out.
The host (numpy) performs the layout/prescale on the way in, and on the
way out applies the closed-form e^t/c_s post-scale plus the exact
cross-chunk boundary corrections (boundary states computed host-side in
fp64 from per-chunk weighted sums; all O(B*T*K) elementwise work, the
O(B*T*K*C) convolution itself runs on device).  bf16 I/O halves HBM
traffic; validated end-to-end rel err ~5e-3 (gate 2e-2).

Sharding: data-parallel over batch, 8 batches per core x 8 cores.
"""

import sys

for _p in ("/opt/trn_rl_repo",):
    if _p not in sys.path:
        sys.path.insert(0, _p)

from contextlib import ExitStack

import ml_dtypes
import numpy as np

import concourse.bacc as bacc
import concourse.bass as bass
import concourse.tile as tile
from concourse import mybir
from concourse.bass_utils import run_bass_kernel_spmd

DT = 0.001
B, T, K = 64, 1024, 512
N_CORES = 8
B_LOC = B // N_CORES  # 8 batches per core
P = 128               # chunk length == partition count
NC = T // P           # 8 time chunks
COLS = NC * K         # 4096 free columns per batch
HALF = COLS // 2      # 2048-col PSUM tiles (4 banks)
QUAD = 512            # one PSUM bank / max moving free dim

F32 = mybir.dt.float32
BF16 = mybir.dt.bfloat16
MULT = mybir.AluOpType.mult

BF = ml_dtypes.bfloat16


def build_nc():
    nc = bacc.Bacc(None, target_bir_lowering=False)

    xs = nc.dram_tensor("xs", [B_LOC, P, COLS], BF16, kind="ExternalInput")
    ba = nc.dram_tensor("ba", [P, K], BF16, kind="ExternalInput")
    w1 = nc.dram_tensor("w1", [P, P], BF16, kind="ExternalInput")
    w2 = nc.dram_tensor("w2", [P, P], BF16, kind="ExternalInput")
    ys = nc.dram_tensor("ys", [B_LOC, P, COLS], BF16, kind="ExternalOutput")

    with tile.TileContext(nc) as tc, ExitStack() as ctx:
        singles = ctx.enter_context(tc.tile_pool(name="singles", bufs=1))
        inpool = ctx.enter_context(tc.tile_pool(name="inpool", bufs=4))
        u2pool = ctx.enter_context(tc.tile_pool(name="u2pool", bufs=2))
        outpool = ctx.enter_context(tc.tile_pool(name="outpool", bufs=2))
        psum = ctx.enter_context(tc.tile_pool(name="psum", bufs=2, space="PSUM"))

        # one-time constants; ba first on the scalar ring (gates batch 0's
        # DVE multiply), weights on sync after batch 0's first input half
        w1_t = singles.tile([P, P], BF16)
        w2_t = singles.tile([P, P], BF16)
        ba_t = singles.tile([P, K], BF16)
        nc.scalar.dma_start(out=ba_t[:], in_=ba[:])

        # PE warm-up during the initial DMA window: HAM needs ~3.4us of
        # activity before the PE clock reaches full rate.
        scratch = singles.tile([P, P], BF16)
        nc.gpsimd.memset(scratch[:], 0.0)
        warm = psum.tile([P, HALF], F32, name="warm", tag="ph")
        for _ in range(6):
            nc.tensor.matmul(warm[:, 0:P], scratch[:], scratch[:], start=True, stop=True)

        def ba_bcast(nchunk):
            a = ba_t[:]
            return bass.AP(
                tensor=a.tensor, offset=a.offset, ap=[a.ap[0], [0, nchunk], [1, K]]
            )

        def as3d(a, nchunk):
            return bass.AP(
                tensor=a.tensor, offset=a.offset, ap=[a.ap[0], [K, nchunk], [1, K]]
            )

        for b in range(B_LOC):
            U = inpool.tile([P, COLS], BF16)
            for h in range(2):
                eng = (nc.sync, nc.scalar)[h]
                eng.dma_start(
                    out=U[:, h * HALF : (h + 1) * HALF],
                    in_=xs[b][:, h * HALF : (h + 1) * HALF],
                )
            if b == 0:
                nc.sync.dma_start(out=w1_t[:], in_=w1[:])
                nc.sync.dma_start(out=w2_t[:], in_=w2[:])
            U2 = u2pool.tile([P, COLS], BF16)

            Y = outpool.tile([P, COLS], BF16)
            for h in range(2):
                base = h * HALF
                nc.vector.tensor_tensor(
                    out=as3d(U2[:, base : base + HALF], HALF // K),
                    in0=as3d(U[:, base : base + HALF], HALF // K),
                    in1=ba_bcast(HALF // K),
                    op=MULT,
                )
                Ph = psum.tile([P, HALF], F32, tag="ph")
                for q in range(4):
                    nc.tensor.matmul(
                        Ph[:, q * QUAD : (q + 1) * QUAD],
                        w1_t[:],
                        U[:, base + q * QUAD : base + (q + 1) * QUAD],
                        start=True,
                        stop=False,
                    )
                for q in range(4):
                    nc.tensor.matmul(
                        Ph[:, q * QUAD : (q + 1) * QUAD],
                        w2_t[:],
                        U2[:, base + q * QUAD : base + (q + 1) * QUAD],
                        start=False,
                        stop=True,
                    )
                nc.scalar.copy(Y[:, base : base + HALF], Ph[:])
                eng = (nc.sync, nc.scalar)[h]
                eng.dma_start(
                    out=ys[b][:, base : base + HALF],
                    in_=Y[:, base : base + HALF],
                )

    nc.compile()
    return nc


_CACHE = {}
PROFILE = False
LAST_RESULT = None


def _constants(initial_level, tau):
    tau_c = np.maximum(tau.astype(np.float64), 1e-8)
    a = DT / tau_c
    e = np.exp(-a)
    c_e = e * a * (1.0 - e)
    c_s = (1.0 - e) - e * a
    return e, c_e, c_s


def kernel(inputs, initial_level, tau):
    global LAST_RESULT
    inputs = np.asarray(inputs, dtype=np.float32)
    initial_level = np.asarray(initial_level, dtype=np.float32)
    tau = np.asarray(tau, dtype=np.float32)
    assert inputs.shape == (B, T, K), inputs.shape

    e, c_e, c_s = _constants(initial_level, tau)
    BA = c_e / (e * c_s)
    j = np.arange(P)
    e_mj = (e[None, :] ** (-j[:, None])).astype(np.float32)   # [P, K]
    e_t = e[None, :] ** (j[:, None])                          # [P, K] fp64

    # ---- prescale + chunk layout: xs[b, j, c*K+k] = u[b, c*P+j, k]*e^-j
    u4 = inputs.reshape(B, NC, P, K)
    xs = np.ascontiguousarray(
        (u4 * e_mj[None, None]).transpose(0, 2, 1, 3)
    ).reshape(B, P, COLS).astype(BF)

    # ---- boundary states (host, fp64; exact cross-chunk stitching) ----
    ws = e[None, :] ** (P - 1 - j[:, None])                   # [P, K]
    weta = (P - 1 - j)[:, None] * e[None, :] ** np.minimum(
        P - 2 - j[:, None], P
    )  # (P-1-j)*e^{P-2-j}; j=P-1 term is zero anyway
    weta[P - 1, :] = 0.0
    inj_s = np.einsum("jk,bcjk->bck", ws, u4, optimize=True)
    inj_eta = np.einsum("jk,bcjk->bck", weta, u4, optimize=True)
    eC = e**P
    s_b = np.broadcast_to(initial_level.astype(np.float64) / (1 - e), (B, K)).copy()
    eta_b = np.broadcast_to(
        initial_level.astype(np.float64) / (1 - e) ** 2, (B, K)
    ).copy()
    s_all = np.empty((B, NC, K))
    eta_all = np.empty((B, NC, K))
    for c in range(NC):
        s_all[:, c] = s_b
        eta_all[:, c] = eta_b
        s_new = eC[None, :] * s_b + inj_s[:, c]
        eta_b = eC[None, :] * eta_b + P * (e ** (P - 1))[None, :] * s_b + inj_eta[:, c]
        s_b = s_new

    # ---- device constants ----
    ba_mat = np.tile(BA.astype(BF)[None, :], (P, 1))          # [P, K]
    tri1 = (j[:, None] <= j[None, :]).astype(BF)              # W1[j, t]
    tri2 = (np.maximum(j[None, :] - j[:, None], 0)).astype(BF)  # W2[j, t]

    if "nc" not in _CACHE:
        _CACHE["nc"] = build_nc()
    nc = _CACHE["nc"]

    in_maps = [
        {
            "xs": xs[i * B_LOC : (i + 1) * B_LOC],
            "ba": ba_mat,
            "w1": tri1,
            "w2": tri2,
        }
        for i in range(N_CORES)
    ]
    res = run_bass_kernel_spmd(nc, in_maps, list(range(N_CORES)), trace=PROFILE)
    LAST_RESULT = res
    P_dev = np.concatenate([np.asarray(r["ys"]) for r in res.results], axis=0)

    # ---- host post: y = e^t*(c_s*P + c_e*e*eta_b) + t1*s_b ----
    Pf = (
        P_dev.astype(np.float32)
        .reshape(B, P, NC, K)
        .transpose(0, 2, 1, 3)
    )  # [B, NC, P(t), K]
    e_t32 = e_t.astype(np.float32)
    t1 = ((c_s * e)[None, :] * e_t + c_e[None, :] * (j + 1)[:, None] * e_t).astype(
        np.float32
    )  # [P, K]
    cee = (c_e * e)[None, :] * e_t                             # [P, K] fp64
    y = (
        e_t32[None, None] * (c_s.astype(np.float32)[None, None, None] * Pf)
        + cee.astype(np.float32)[None, None] * eta_all.astype(np.float32)[:, :, None, :]
        + t1[None, None] * s_all.astype(np.float32)[:, :, None, :]
    )
    return np.ascontiguousarray(y.reshape(B, T, K), dtype=np.float32)


# revision 9
# speedup vs baseline: 1.1399x; 1.0097x over previous
"""Alpha-filter (keras_spiking AlphaCell) Trainium2 Bass kernel.

Math: per (batch b, feature k) the reference runs the 2-state recurrence
    x_t = A_k x_{t-1} + B_k u_t,   y_t = x_t[1]
with A_k = e*[[1-a, -a/tau],[dt, 1+a]], a = dt/tau, e = exp(-a), which
reduces to two chained first-order scans (defective double pole at e):
    s_t = e s_{t-1} + u_t;  eta_t = e eta_{t-1} + s_{t-1};
    y_t = c_e eta_t + c_s s_t,   c_e = e*a*(1-e), c_s = (1-e)-e*a.

Instead of running the scans on the (slow, ~2.2 cycles/element) DVE scan
unit, time is split into chunks of C=128 and the chunk-local response is
computed on the PE as two shared-weight triangular matmuls via an
exponential rescale: with uhat_j = u_j * e^{-j} (per-feature prescale),

    P_t = sum_{j<=t} uhat_j + BA * sum_{j<=t} (t-j) uhat_j,
    y_local_t = c_s e^t P_t,           BA = c_e/(e c_s)

where the two lower-triangular weight matrices (ones / ramp) are shared
by every feature.  The device computes P for all chunks:
  DMA-in (bf16) -> DVE: U2 = U*BA (one 2-byte 2x op) ->
  PE: P = W1^T@U + W2^T@U2 (PSUM fp32, 512-col banks) ->
  Scalar: PSUM->SBUF bf16 -> DMA-out.
The host (numpy) performs the layout/prescale on the way in, and on the
way out applies the closed-form e^t/c_s post-scale plus the exact
cross-chunk boundary corrections (boundary states computed host-side in
fp64 from per-chunk weighted sums; all O(B*T*K) elementwise work, the
O(B*T*K*C) convolution itself runs on device).  bf16 I/O halves HBM
traffic; validated end-to-end rel err ~5e-3 (gate 2e-2).

Sharding: data-parallel over batch, 8 batches per core x 8 cores.
"""

import sys

for _p in ("/opt/trn_rl_repo",):
    if _p not in sys.path:
        sys.path.insert(0, _p)

from contextlib import ExitStack

import ml_dtypes
import numpy as np

import concourse.bacc as bacc
import concourse.bass as bass
import concourse.tile as tile
from concourse import mybir
from concourse.bass_utils import run_bass_kernel_spmd

DT = 0.001
B, T, K = 64, 1024, 512
N_CORES = 8
B_LOC = B // N_CORES  # 8 batches per core
P = 128               # chunk length == partition count
NC = T // P           # 8 time chunks
COLS = NC * K         # 4096 free columns per batch
HALF = COLS // 2      # 2048-col PSUM tiles (4 banks)
QUAD = 512            # one PSUM bank / max moving free dim

F32 = mybir.dt.float32
BF16 = mybir.dt.bfloat16
MULT = mybir.AluOpType.mult

BF = ml_dtypes.bfloat16


def build_nc():
    nc = bacc.Bacc(None, target_bir_lowering=False)

    xs = nc.dram_tensor("xs", [B_LOC, P, COLS], BF16, kind="ExternalInput")
    ba = nc.dram_tensor("ba", [P, K], BF16, kind="ExternalInput")
    w1 = nc.dram_tensor("w1", [P, P], BF16, kind="ExternalInput")
    w2 = nc.dram_tensor("w2", [P, P], BF16, kind="ExternalInput")
    ys = nc.dram_tensor("ys", [B_LOC, P, COLS], BF16, kind="ExternalOutput")

    with tile.TileContext(nc) as tc, ExitStack() as ctx:
        singles = ctx.enter_context(tc.tile_pool(name="singles", bufs=1))
        inpool = ctx.enter_context(tc.tile_pool(name="inpool", bufs=4))
        u2pool = ctx.enter_context(tc.tile_pool(name="u2pool", bufs=2))
        outpool = ctx.enter_context(tc.tile_pool(name="outpool", bufs=2))
        psum = ctx.enter_context(tc.tile_pool(name="psum", bufs=2, space="PSUM"))

        # one-time constants; ba first on the scalar ring (gates batch 0's
        # DVE multiply), weights on sync after batch 0's first input half
        w1_t = singles.tile([P, P], BF16)
        w2_t = singles.tile([P, P], BF16)
        ba_t = singles.tile([P, K], BF16)
        nc.scalar.dma_start(out=ba_t[:], in_=ba[:])

        # PE warm-up during the initial DMA window: HAM needs ~3.4us of
        # activity before the PE clock reaches full rate.
        scratch = singles.tile([P, P], BF16)
        nc.gpsimd.memset(scratch[:], 0.0)
        warm = psum.tile([P, HALF], F32, name="warm", tag="ph")
        for _ in range(6):
            nc.tensor.matmul(warm[:, 0:P], scratch[:], scratch[:], start=True, stop=True)

        def ba_bcast(nchunk):
            a = ba_t[:]
            return bass.AP(
                tensor=a.tensor, offset=a.offset, ap=[a.ap[0], [0, nchunk], [1, K]]
            )

        def as3d(a, nchunk):
            return bass.AP(
                tensor=a.tensor, offset=a.offset, ap=[a.ap[0], [K, nchunk], [1, K]]
            )

        for b in range(B_LOC):
            U = inpool.tile([P, COLS], BF16)
            for h in range(2):
                eng = (nc.sync, nc.scalar)[h]
                eng.dma_start(
                    out=U[:, h * HALF : (h + 1) * HALF],
                    in_=xs[b][:, h * HALF : (h + 1) * HALF],
                )
            if b == 0:
                nc.sync.dma_start(out=w1_t[:], in_=w1[:])
                nc.sync.dma_start(out=w2_t[:], in_=w2[:])
            U2 = u2pool.tile([P, COLS], BF16)

            Y = outpool.tile([P, COLS], BF16)
            for h in range(2):
                base = h * HALF
                nc.vector.tensor_tensor(
                    out=as3d(U2[:, base : base + HALF], HALF // K),
                    in0=as3d(U[:, base : base + HALF], HALF // K),
                    in1=ba_bcast(HALF // K),
                    op=MULT,
                )
                Ph = psum.tile([P, HALF], F32, tag="ph")
                for q in range(4):
                    nc.tensor.matmul(
                        Ph[:, q * QUAD : (q + 1) * QUAD],
                        w1_t[:],
                        U[:, base + q * QUAD : base + (q + 1) * QUAD],
                        start=True,
                        stop=False,
                    )
                for q in range(4):
                    nc.tensor.matmul(
                        Ph[:, q * QUAD : (q + 1) * QUAD],
                        w2_t[:],
                        U2[:, base + q * QUAD : base + (q + 1) * QUAD],
                        start=False,
                        stop=True,
                    )
                # scalar handles early copies; late batches use the DVE,
                # which has gone idle (all U2 multiplies done) by then
                if b < 5:
                    nc.scalar.copy(Y[:, base : base + HALF], Ph[:])
                else:
                    nc.vector.tensor_copy(Y[:, base : base + HALF], Ph[:])
                eng = (nc.sync, nc.scalar)[h]
                eng.dma_start(
                    out=ys[b][:, base : base + HALF],
                    in_=Y[:, base : base + HALF],
                )

    nc.compile()
    return nc


_CACHE = {}
PROFILE = False
LAST_RESULT = None


def _constants(initial_level, tau):
    tau_c = np.maximum(tau.astype(np.float64), 1e-8)
    a = DT / tau_c
    e = np.exp(-a)
    c_e = e * a * (1.0 - e)
    c_s = (1.0 - e) - e * a
    return e, c_e, c_s


def kernel(inputs, initial_level, tau):
    global LAST_RESULT
    inputs = np.asarray(inputs, dtype=np.float32)
    initial_level = np.asarray(initial_level, dtype=np.float32)
    tau = np.asarray(tau, dtype=np.float32)
    assert inputs.shape == (B, T, K), inputs.shape

    e, c_e, c_s = _constants(initial_level, tau)
    BA = c_e / (e * c_s)
    j = np.arange(P)
    e_mj = (e[None, :] ** (-j[:, None])).astype(np.float32)   # [P, K]
    e_t = e[None, :] ** (j[:, None])                          # [P, K] fp64

    # ---- prescale + chunk layout: xs[b, j, c*K+k] = u[b, c*P+j, k]*e^-j
    u4 = inputs.reshape(B, NC, P, K)
    xs = np.ascontiguousarray(
        (u4 * e_mj[None, None]).transpose(0, 2, 1, 3)
    ).reshape(B, P, COLS).astype(BF)

    # ---- boundary states (host, fp64; exact cross-chunk stitching) ----
    ws = e[None, :] ** (P - 1 - j[:, None])                   # [P, K]
    weta = (P - 1 - j)[:, None] * e[None, :] ** np.minimum(
        P - 2 - j[:, None], P
    )  # (P-1-j)*e^{P-2-j}; j=P-1 term is zero anyway
    weta[P - 1, :] = 0.0
    inj_s = np.einsum("jk,bcjk->bck", ws, u4, optimize=True)
    inj_eta = np.einsum("jk,bcjk->bck", weta, u4, optimize=True)
    eC = e**P
    s_b = np.broadcast_to(initial_level.astype(np.float64) / (1 - e), (B, K)).copy()
    eta_b = np.broadcast_to(
        initial_level.astype(np.float64) / (1 - e) ** 2, (B, K)
    ).copy()
    s_all = np.empty((B, NC, K))
    eta_all = np.empty((B, NC, K))
    for c in range(NC):
        s_all[:, c] = s_b
        eta_all[:, c] = eta_b
        s_new = eC[None, :] * s_b + inj_s[:, c]
        eta_b = eC[None, :] * eta_b + P * (e ** (P - 1))[None, :] * s_b + inj_eta[:, c]
        s_b = s_new

    # ---- device constants ----
    ba_mat = np.tile(BA.astype(BF)[None, :], (P, 1))          # [P, K]
    tri1 = (j[:, None] <= j[None, :]).astype(BF)              # W1[j, t]
    tri2 = (np.maximum(j[None, :] - j[:, None], 0)).astype(BF)  # W2[j, t]

    if "nc" not in _CACHE:
        _CACHE["nc"] = build_nc()
    nc = _CACHE["nc"]

    in_maps = [
        {
            "xs": xs[i * B_LOC : (i + 1) * B_LOC],
            "ba": ba_mat,
            "w1": tri1,
            "w2": tri2,
        }
        for i in range(N_CORES)
    ]
    res = run_bass_kernel_spmd(nc, in_maps, list(range(N_CORES)), trace=PROFILE)
    LAST_RESULT = res
    P_dev = np.concatenate([np.asarray(r["ys"]) for r in res.results], axis=0)

    # ---- host post: y = e^t*(c_s*P + c_e*e*eta_b) + t1*s_b ----
    Pf = (
        P_dev.astype(np.float32)
        .reshape(B, P, NC, K)
        .transpose(0, 2, 1, 3)
    )  # [B, NC, P(t), K]
    e_t32 = e_t.astype(np.float32)
    t1 = ((c_s * e)[None, :] * e_t + c_e[None, :] * (j + 1)[:, None] * e_t).astype(
        np.float32
    )  # [P, K]
    cee = (c_e * e)[None, :] * e_t                             # [P, K] fp64
    y = (
        e_t32[None, None] * (c_s.astype(np.float32)[None, None, None] * Pf)
        + cee.astype(np.float32)[None, None] * eta_all.astype(np.float32)[:, :, None, :]
        + t1[None, None] * s_all.astype(np.float32)[:, :, None, :]
    )
    return np.ascontiguousarray(y.reshape(B, T, K), dtype=np.float32)


# revision 10
# speedup vs baseline: 1.1550x; 1.0133x over previous
"""Alpha-filter (keras_spiking AlphaCell) Trainium2 Bass kernel.

Math: per (batch b, feature k) the reference runs the 2-state recurrence
    x_t = A_k x_{t-1} + B_k u_t,   y_t = x_t[1]
with A_k = e*[[1-a, -a/tau],[dt, 1+a]], a = dt/tau, e = exp(-a), which
reduces to two chained first-order scans (defective double pole at e):
    s_t = e s_{t-1} + u_t;  eta_t = e eta_{t-1} + s_{t-1};
    y_t = c_e eta_t + c_s s_t,   c_e = e*a*(1-e), c_s = (1-e)-e*a.

Instead of running the scans on the (slow, ~2.2 cycles/element) DVE scan
unit, time is split into chunks of C=128 and the chunk-local response is
computed on the PE as two shared-weight triangular matmuls via an
exponential rescale: with uhat_j = u_j * e^{-j} (per-feature prescale),

    P_t = sum_{j<=t} uhat_j + BA * sum_{j<=t} (t-j) uhat_j,
    y_local_t = c_s e^t P_t,           BA = c_e/(e c_s)

where the two lower-triangular weight matrices (ones / ramp) are shared
by every feature.  The device computes P for all chunks:
  DMA-in (bf16) -> DVE: U2 = U*BA (one 2-byte 2x op) ->
  PE: P = W1^T@U + W2^T@U2 (PSUM fp32, 512-col banks) ->
  Scalar: PSUM->SBUF bf16 -> DMA-out.
The host (numpy) performs the layout/prescale on the way in, and on the
way out applies the closed-form e^t/c_s post-scale plus the exact
cross-chunk boundary corrections (boundary states computed host-side in
fp64 from per-chunk weighted sums; all O(B*T*K) elementwise work, the
O(B*T*K*C) convolution itself runs on device).  bf16 I/O halves HBM
traffic; validated end-to-end rel err ~5e-3 (gate 2e-2).

Sharding: data-parallel over batch, 8 batches per core x 8 cores.
"""

import sys

for _p in ("/opt/trn_rl_repo",):
    if _p not in sys.path:
        sys.path.insert(0, _p)

from contextlib import ExitStack

import ml_dtypes
import numpy as np

import concourse.bacc as bacc
import concourse.bass as bass
import concourse.tile as tile
from concourse import mybir
from concourse.bass_utils import run_bass_kernel_spmd

DT = 0.001
B, T, K = 64, 1024, 512
N_CORES = 8
B_LOC = B // N_CORES  # 8 batches per core
P = 128               # chunk length == partition count
NC = T // P           # 8 time chunks
COLS = NC * K         # 4096 free columns per batch
HALF = COLS // 2      # 2048-col PSUM tiles (4 banks)
QUAD = 512            # one PSUM bank / max moving free dim

F32 = mybir.dt.float32
BF16 = mybir.dt.bfloat16
MULT = mybir.AluOpType.mult

BF = ml_dtypes.bfloat16


def build_nc():
    nc = bacc.Bacc(None, target_bir_lowering=False)

    xs = nc.dram_tensor("xs", [B_LOC, P, COLS], BF16, kind="ExternalInput")
    ba = nc.dram_tensor("ba", [P, K], BF16, kind="ExternalInput")
    w1 = nc.dram_tensor("w1", [P, P], BF16, kind="ExternalInput")
    w2 = nc.dram_tensor("w2", [P, P], BF16, kind="ExternalInput")
    ys = nc.dram_tensor("ys", [B_LOC, P, COLS], BF16, kind="ExternalOutput")

    with tile.TileContext(nc) as tc, ExitStack() as ctx:
        singles = ctx.enter_context(tc.tile_pool(name="singles", bufs=1))
        inpool = ctx.enter_context(tc.tile_pool(name="inpool", bufs=4))
        u2pool = ctx.enter_context(tc.tile_pool(name="u2pool", bufs=2))
        outpool = ctx.enter_context(tc.tile_pool(name="outpool", bufs=2))
        psum = ctx.enter_context(tc.tile_pool(name="psum", bufs=2, space="PSUM"))

        # one-time constants; ba first on the scalar ring (gates batch 0's
        # DVE multiply), weights on sync after batch 0's first input half
        w1_t = singles.tile([P, P], BF16)
        w2_t = singles.tile([P, P], BF16)
        ba_t = singles.tile([P, K], BF16)
        nc.scalar.dma_start(out=ba_t[:], in_=ba[:])

        # PE warm-up during the initial DMA window: HAM needs ~3.4us of
        # activity before the PE clock reaches full rate.
        scratch = singles.tile([P, P], BF16)
        nc.gpsimd.memset(scratch[:], 0.0)
        warm = psum.tile([P, HALF], F32, name="warm", tag="ph")
        for _ in range(6):
            nc.tensor.matmul(warm[:, 0:P], scratch[:], scratch[:], start=True, stop=True)

        def ba_bcast(nchunk):
            a = ba_t[:]
            return bass.AP(
                tensor=a.tensor, offset=a.offset, ap=[a.ap[0], [0, nchunk], [1, K]]
            )

        def as3d(a, nchunk):
            return bass.AP(
                tensor=a.tensor, offset=a.offset, ap=[a.ap[0], [K, nchunk], [1, K]]
            )

        # Software-pipelined emission: batch b+1's input DMAs and DVE
        # multiplies are emitted BEFORE batch b's PSUM->SBUF copies, so the
        # engine FIFOs (which follow issue order) keep the PE fed with U2
        # data while copies drain.  Copies split scalar(h0)/DVE(h1) so the
        # output path runs at two engines' rate (~500 GB/s > HBM).
        us = {}
        u2s = {}

        def stage_in(b):
            U = inpool.tile([P, COLS], BF16)
            for h in range(2):
                eng = (nc.sync, nc.scalar)[h]
                eng.dma_start(
                    out=U[:, h * HALF : (h + 1) * HALF],
                    in_=xs[b][:, h * HALF : (h + 1) * HALF],
                )
            us[b] = U
            if b == 0:
                nc.sync.dma_start(out=w1_t[:], in_=w1[:])
                nc.sync.dma_start(out=w2_t[:], in_=w2[:])

        def stage_mul(b):
            U = us[b]
            U2 = u2pool.tile([P, COLS], BF16)
            for h in range(2):
                base = h * HALF
                nc.vector.tensor_tensor(
                    out=as3d(U2[:, base : base + HALF], HALF // K),
                    in0=as3d(U[:, base : base + HALF], HALF // K),
                    in1=ba_bcast(HALF // K),
                    op=MULT,
                )
            u2s[b] = U2

        stage_in(0)
        stage_mul(0)
        stage_in(1)
        for b in range(B_LOC):
            U, U2 = us[b], u2s[b]
            Y = outpool.tile([P, COLS], BF16)
            phs = []
            for h in range(2):
                base = h * HALF
                Ph = psum.tile([P, HALF], F32, tag="ph")
                phs.append(Ph)
                for q in range(4):
                    nc.tensor.matmul(
                        Ph[:, q * QUAD : (q + 1) * QUAD],
                        w1_t[:],
                        U[:, base + q * QUAD : base + (q + 1) * QUAD],
                        start=True,
                        stop=False,
                    )
                for q in range(4):
                    nc.tensor.matmul(
                        Ph[:, q * QUAD : (q + 1) * QUAD],
                        w2_t[:],
                        U2[:, base + q * QUAD : base + (q + 1) * QUAD],
                        start=False,
                        stop=True,
                    )
            # prefetch/precompute the NEXT batch before this batch's copies
            if b + 1 < B_LOC:
                stage_mul(b + 1)
            if b + 2 < B_LOC:
                stage_in(b + 2)
            for h in range(2):
                base = h * HALF
                if h == 0:
                    nc.scalar.copy(Y[:, base : base + HALF], phs[h][:])
                else:
                    nc.vector.tensor_copy(Y[:, base : base + HALF], phs[h][:])
                eng = (nc.sync, nc.scalar)[h]
                eng.dma_start(
                    out=ys[b][:, base : base + HALF],
                    in_=Y[:, base : base + HALF],
                )

    nc.compile()
    return nc


_CACHE = {}
PROFILE = False
LAST_RESULT = None


def _constants(initial_level, tau):
    tau_c = np.maximum(tau.astype(np.float64), 1e-8)
    a = DT / tau_c
    e = np.exp(-a)
    c_e = e * a * (1.0 - e)
    c_s = (1.0 - e) - e * a
    return e, c_e, c_s


def kernel(inputs, initial_level, tau):
    global LAST_RESULT
    inputs = np.asarray(inputs, dtype=np.float32)
    initial_level = np.asarray(initial_level, dtype=np.float32)
    tau = np.asarray(tau, dtype=np.float32)
    assert inputs.shape == (B, T, K), inputs.shape

    e, c_e, c_s = _constants(initial_level, tau)
    BA = c_e / (e * c_s)
    j = np.arange(P)
    e_mj = (e[None, :] ** (-j[:, None])).astype(np.float32)   # [P, K]
    e_t = e[None, :] ** (j[:, None])                          # [P, K] fp64

    # ---- prescale + chunk layout: xs[b, j, c*K+k] = u[b, c*P+j, k]*e^-j
    u4 = inputs.reshape(B, NC, P, K)
    xs = np.ascontiguousarray(
        (u4 * e_mj[None, None]).transpose(0, 2, 1, 3)
    ).reshape(B, P, COLS).astype(BF)

    # ---- boundary states (host, fp64; exact cross-chunk stitching) ----
    ws = e[None, :] ** (P - 1 - j[:, None])                   # [P, K]
    weta = (P - 1 - j)[:, None] * e[None, :] ** np.minimum(
        P - 2 - j[:, None], P
    )  # (P-1-j)*e^{P-2-j}; j=P-1 term is zero anyway
    weta[P - 1, :] = 0.0
    inj_s = np.einsum("jk,bcjk->bck", ws, u4, optimize=True)
    inj_eta = np.einsum("jk,bcjk->bck", weta, u4, optimize=True)
    eC = e**P
    s_b = np.broadcast_to(initial_level.astype(np.float64) / (1 - e), (B, K)).copy()
    eta_b = np.broadcast_to(
        initial_level.astype(np.float64) / (1 - e) ** 2, (B, K)
    ).copy()
    s_all = np.empty((B, NC, K))
    eta_all = np.empty((B, NC, K))
    for c in range(NC):
        s_all[:, c] = s_b
        eta_all[:, c] = eta_b
        s_new = eC[None, :] * s_b + inj_s[:, c]
        eta_b = eC[None, :] * eta_b + P * (e ** (P - 1))[None, :] * s_b + inj_eta[:, c]
        s_b = s_new

    # ---- device constants ----
    ba_mat = np.tile(BA.astype(BF)[None, :], (P, 1))          # [P, K]
    tri1 = (j[:, None] <= j[None, :]).astype(BF)              # W1[j, t]
    tri2 = (np.maximum(j[None, :] - j[:, None], 0)).astype(BF)  # W2[j, t]

    if "nc" not in _CACHE:
        _CACHE["nc"] = build_nc()
    nc = _CACHE["nc"]

    in_maps = [
        {
            "xs": xs[i * B_LOC : (i + 1) * B_LOC],
            "ba": ba_mat,
            "w1": tri1,
            "w2": tri2,
        }
        for i in range(N_CORES)
    ]
    res = run_bass_kernel_spmd(nc, in_maps, list(range(N_CORES)), trace=PROFILE)
    LAST_RESULT = res
    P_dev = np.concatenate([np.asarray(r["ys"]) for r in res.results], axis=0)

    # ---- host post: y = e^t*(c_s*P + c_e*e*eta_b) + t1*s_b ----
    Pf = (
        P_dev.astype(np.float32)
        .reshape(B, P, NC, K)
        .transpose(0, 2, 1, 3)
    )  # [B, NC, P(t), K]
    e_t32 = e_t.astype(np.float32)
    t1 = ((c_s * e)[None, :] * e_t + c_e[None, :] * (j + 1)[:, None] * e_t).astype(
        np.float32
    )  # [P, K]
    cee = (c_e * e)[None, :] * e_t                             # [P, K] fp64
    y = (
        e_t32[None, None] * (c_s.astype(np.float32)[None, None, None] * Pf)
        + cee.astype(np.float32)[None, None] * eta_all.astype(np.float32)[:, :, None, :]
        + t1[None, None] * s_all.astype(np.float32)[:, :, None, :]
    )
    return np.ascontiguousarray(y.reshape(B, T, K), dtype=np.float32)
